# revision 78
# baseline (speedup 1.0000x reference)
"""Trainium2 Bass kernel for DenseLanguageGuidanceModule.

Math (per batch b):
    fk_l = fl @ W_lk + b_lk            [77, 512]
    fv-side projections are folded away algebraically:
      a_raw = (fk_l @ W_vk^T) @ fv^T + (fk_l @ b_vk) 1^T   (/= sqrt(512))
      fa_v  = diag(1/s1) (E @ fv) @ W_vv + b_vv,  E = exp(a_raw/sqrt(512))
      fm    = diag(1/s2) E^T @ (fv_l @ fa_v^T)
      out   = fm @ W_m + b_m
    where s1 = row sums of E, s2 = column sums of E.

Distribution: pure data-parallel over batch B=32 across 8 NeuronCores
(4 batches per core), weights replicated. No collectives.

All matmuls run in float32r (TF32-like: 11 mantissa bits, fp32 accumulate)
which is full PE speed for free-dim >= 256. Inputs are pre-rounded to f32r
on the host (RNE, keep top 20 bits) so on-device rounding is a no-op.
"""
import sys

sys.path.insert(0, "/opt/trn_rl_repo")

import numpy as np

import concourse.bass as bass  # noqa: E402
import concourse.tile as tile  # noqa: E402
from concourse import bacc, mybir  # noqa: E402
from concourse.bass_utils import run_bass_kernel_spmd  # noqa: E402

P = 128
NCORES = 8
B = 32
BL = B // NCORES          # 4 batches per core
NV, DV = 1024, 768        # vision tokens / dim
NL, DL = 77, 512          # language tokens / dim
D = 512                   # shared feature dim
OD = 768                  # output dim
NLB = NL * BL             # 308: l-dim stacked across local batches

F32R = mybir.dt.float32r
F32 = mybir.dt.float32
ISQD = 1.0 / float(np.sqrt(np.float32(D)))

AF = mybir.ActivationFunctionType


def round_f32r(x: np.ndarray) -> np.ndarray:
    """RNE-round fp32 to f32r (1s+8e+11m in the top 20 bits)."""
    u = np.ascontiguousarray(x, dtype=np.float32).view(np.uint32)
    low = u & np.uint32(0xFFF)
    base = u & np.uint32(0xFFFFF000)
    lsb = (u >> np.uint32(12)) & np.uint32(1)
    up = (low > 0x800) | ((low == 0x800) & (lsb == 1))
    return (base + np.where(up, np.uint32(0x1000), np.uint32(0))).view(np.float32)


def _build():
    nc = bacc.Bacc("TRN2", target_bir_lowering=False)

    fv_d = nc.dram_tensor("fv", [BL, NV, DV], F32R, kind="ExternalInput")
    fl_d = nc.dram_tensor("fl", [BL, NL, DL], F32R, kind="ExternalInput")
    wlk_d = nc.dram_tensor("wlk", [DL, D], F32R, kind="ExternalInput")
    wlv_d = nc.dram_tensor("wlv", [DL, D], F32R, kind="ExternalInput")
    wvkT_d = nc.dram_tensor("wvkT", [D, DV], F32R, kind="ExternalInput")
    wvv_d = nc.dram_tensor("wvv", [DV, D], F32R, kind="ExternalInput")
    wm_d = nc.dram_tensor("wm", [97, OD], F32R, kind="ExternalInput")
    blk_d = nc.dram_tensor("blk", [DL], F32, kind="ExternalInput")
    blv_d = nc.dram_tensor("blv", [DL], F32, kind="ExternalInput")
    bvv_d = nc.dram_tensor("bvv", [D], F32, kind="ExternalInput")
    bvk_d = nc.dram_tensor("bvk", [D], F32R, kind="ExternalInput")
    iden_d = nc.dram_tensor("iden", [P, P], F32R, kind="ExternalInput")
    onesr_d = nc.dram_tensor("onesr", [1, 512], F32R, kind="ExternalInput")
    onesc_d = nc.dram_tensor("onesc", [P, 1], F32R, kind="ExternalInput")
    zeros_d = nc.dram_tensor("zeros", [19, NV], F32R, kind="ExternalInput")
    out_d = nc.dram_tensor("out", [BL, NV, OD], F32, kind="ExternalOutput")

    with tile.TileContext(nc) as tc:
        with (
            tc.tile_pool(name="consts", bufs=1) as cp,
            tc.tile_pool(name="lph", bufs=1) as lp,
            tc.tile_pool(name="fvn", bufs=2) as fvnp,
            tc.tile_pool(name="fvt", bufs=3) as fvtp,
            tc.tile_pool(name="eb", bufs=2) as ebp,
            tc.tile_pool(name="sm", bufs=2) as smp,
            tc.tile_pool(name="outp", bufs=4) as outp,
            tc.tile_pool(name="tp", bufs=4, space="PSUM") as tp,       # 1-bank slots
            tc.tile_pool(name="acc", bufs=2, space="PSUM") as accp,    # 2-bank slots
        ):
            # ---------------- constants (early: identity only) ----------------
            iden = cp.tile([P, P], F32R)
            nc.sync.dma_start(iden, iden_d[:, :])
            # ---------------- language phase (batched over BL) ----------------
            lph_tmp = tc.tile_pool(name="lphtmp", bufs=1)
            lpt = lph_tmp.__enter__()
            # FLT = fl_all^T  [512(D1 on p), 308]
            FLT = lpt.tile([P, 4, NLB], F32R)
            with tc.tile_pool(name="fln", bufs=1) as flnp:
                fl_flat = fl_d.rearrange("b l d -> (b l) d")
                row_tiles = [(0, P), (P, P), (2 * P, NLB - 2 * P)]
                for half in range(2):
                    c0 = half * 256
                    FLn = flnp.tile([P, 3, 256], F32R, tag="fln")
                    for i, (r0, sz) in enumerate(row_tiles):
                        nc.sync.dma_start(
                            FLn[:sz, i, :], fl_flat[r0 : r0 + sz, c0 : c0 + 256]
                        )
                    for fb in range(2):
                        ps = tp.tile([P, 384], F32R, tag="tp")
                        for i in range(3):
                            nc.tensor.transpose(
                                ps[:, i * P : (i + 1) * P],
                                FLn[:, i, fb * P : (fb + 1) * P],
                                iden,
                            )
                        fbo = half * 2 + fb
                        nc.vector.tensor_copy(FLT[:, fbo, : 2 * P], ps[:, : 2 * P])
                        nc.vector.tensor_copy(
                            FLT[:, fbo, 2 * P :], ps[:, 2 * P : 2 * P + (NLB - 2 * P)]
                        )

            # ---------------- remaining constants, interleaved with fv[0] ----------------
            FVn0 = fvnp.tile([P, 8, DV], F32R, tag="fvn")
            fvb0 = fv_d[0].rearrange("(t p) d -> p t d", p=P)
            Wlk = lpt.tile([P, 4, D], F32R)
            nc.sync.dma_start(Wlk, wlk_d.rearrange("(ko p) m -> p ko m", p=P))
            blk = cp.tile([P, 4], F32)
            nc.sync.dma_start(blk, blk_d.rearrange("(ko p) -> p ko", p=P))
            Wlv = lpt.tile([P, 4, D], F32R)
            nc.sync.dma_start(Wlv, wlv_d.rearrange("(ko p) m -> p ko m", p=P))
            blv = cp.tile([P, 4], F32)
            nc.sync.dma_start(blv, blv_d.rearrange("(ko p) -> p ko", p=P))

            WvkT = cp.tile([P, 4, DV], F32R)
            nc.sync.dma_start(WvkT, wvkT_d.rearrange("(ko p) m -> p ko m", p=P))
            bvk = cp.tile([P, 4], F32R)
            nc.sync.dma_start(bvk, bvk_d.rearrange("(ko p) -> p ko", p=P))
            for q in range(4):
                nc.sync.dma_start(FVn0[:, 2 * q : 2 * q + 2, :], fvb0[:, 2 * q : 2 * q + 2, :])
            onesr = cp.tile([1, 512], F32R)
            nc.sync.dma_start(onesr, onesr_d[:, :])
            onesc = cp.tile([P, 1], F32R)
            nc.sync.dma_start(onesc, onesc_d[:, :])


            # FKL = fk_l^T (+b_lk), FVL = fv_l^T (+b_lv): [512, 308]
            FKL = lpt.tile([P, 4, NLB], F32R)
            FVL = lp.tile([P, 4, NLB], F32R)
            for W, bias, OUT in ((Wlk, blk, FKL), (Wlv, blv, FVL)):
                for m in range(4):
                    ps = tp.tile([P, NLB], F32, tag="tp")
                    for ko in range(4):
                        nc.tensor.matmul(
                            ps, W[:, ko, m * P : (m + 1) * P], FLT[:, ko, :],
                            start=(ko == 0), stop=(ko == 3),
                        )
                    nc.vector.tensor_scalar_add(OUT[:, m, :], ps, bias[:, m, None])

            # GT = g^T = W_vk @ fk_l^T : [768, 308]
            GT = lp.tile([P, 6, NLB], F32R)
            for mv in range(6):
                ps = tp.tile([P, NLB], F32, tag="tp")
                for ko in range(4):
                    nc.tensor.matmul(
                        ps, WvkT[:, ko, mv * P : (mv + 1) * P], FKL[:, ko, :],
                        start=(ko == 0), stop=(ko == 3),
                    )
                if mv % 2 == 0:
                    nc.vector.tensor_copy(GT[:, mv, :], ps)
                else:
                    nc.scalar.activation(GT[:, mv, :], ps, AF.Copy)

            # C = (fk_l @ b_vk)^T : [1, 308]
            Cst = lp.tile([1, NLB], F32R)
            psc = tp.tile([1, NLB], F32, tag="tp")
            for ko in range(4):
                nc.tensor.matmul(
                    psc, bvk[:, ko, None], FKL[:, ko, :],
                    start=(ko == 0), stop=(ko == 3),
                )
            nc.vector.tensor_copy(Cst, psc)
            lph_tmp.__exit__(None, None, None)
            fvn3_pool = tc.tile_pool(name="fvn3", bufs=1)
            fvn3 = fvn3_pool.__enter__()

            # late-use constants (not needed until fa_v / final)
            Wvv = cp.tile([P, 6, D], F32R)
            nc.sync.dma_start(Wvv, wvv_d.rearrange("(ko p) m -> p ko m", p=P))
            Wm = cp.tile([97, OD], F32R)
            nc.sync.dma_start(Wm, wm_d[:, :])
            bvv = cp.tile([P, 4], F32)
            nc.sync.dma_start(bvv, bvv_d.rearrange("(ko p) -> p ko", p=P))


            # persistent FMT ping-pong pair; filler rows 77..95 zeroed once
            FMTs = []
            for _i in range(2):
                _f = lp.tile([97, NV], F32R, tag=f"FMT{_i}")
                nc.sync.dma_start(_f[NL:96, :], zeros_d[:, :])
                FMTs.append(_f)

            # ---------------- per-batch vision phase ----------------
            pending_finals = []
            vstate = {}

            def _emit_tg(FVn, tg):
                fvth = fvtp.tile([P, 6, 512], F32R, tag="fvt")
                for ko in range(6):
                    ps = tp.tile([P, 512], F32R, tag="tp")
                    for tt in range(4):
                        t = tg * 4 + tt
                        nc.tensor.transpose(
                            ps[:, tt * P : (tt + 1) * P],
                            FVn[:, t, ko * P : (ko + 1) * P],
                            iden,
                        )
                    if (ko + tg) % 2 == 0:
                        nc.vector.tensor_copy(fvth[:, ko, :], ps)
                    else:
                        nc.scalar.activation(fvth[:, ko, :], ps, AF.Copy)
                return fvth

            def _emit_load_tg0(nb):
                if nb == 0:
                    FVn = FVn0
                else:
                    pool_b = fvn3 if nb == 2 else fvnp
                    FVn = pool_b.tile([P, 8, DV], F32R, tag="fvn")
                    fvb = fv_d[nb].rearrange("(t p) d -> p t d", p=P)
                    for q in range(4):
                        nc.sync.dma_start(
                            FVn[:, 2 * q : 2 * q + 2, :], fvb[:, 2 * q : 2 * q + 2, :]
                        )
                vstate[nb] = [FVn, [_emit_tg(FVn, 0)], None]

            def _emit_araw_E(nb, FVn, FVTh):
                nls = nb * NL
                if len(FVTh) == 1:
                    FVTh.append(_emit_tg(FVn, 1))
                # a_raw = g @ fv^T + c 1^T  -> psum [77, 1024]
                psa = accp.tile([NL, NV], F32, tag="acc")
                for nv in range(2):
                    sl = psa[:, nv * 512 : (nv + 1) * 512]
                    for ko in range(6):
                        nc.tensor.matmul(
                            sl, GT[:, ko, nls : nls + NL],
                            FVTh[nv][:, ko, :],
                            start=(ko == 0), stop=False,
                        )
                    nc.tensor.matmul(
                        sl, Cst[:1, nls : nls + NL],
                        onesr[:1, :512],
                        start=False, stop=True,
                    )
                # E = exp(a/sqrt(D)); s1 = row sums
                E = ebp.tile([P, NV], F32R, tag="E")
                s1p = smp.tile([NL, 2], F32, tag="s1p")
                for nv in range(2):
                    nc.scalar.activation(
                        E[:NL, nv * 512 : (nv + 1) * 512],
                        psa[:, nv * 512 : (nv + 1) * 512],
                        AF.Exp, scale=ISQD, accum_out=s1p[:, nv, None],
                    )
                s1 = smp.tile([NL, 1], F32, tag="s1")
                nc.vector.reduce_sum(s1, s1p, axis=mybir.AxisListType.X)
                ivs1 = smp.tile([NL, 1], F32, tag="ivs1")
                nc.vector.reciprocal(ivs1, s1)
                return (E, ivs1)

            _emit_load_tg0(0)
            for b in range(BL):
                ls = b * NL  # column offset of this batch in *_all tensors

                FVn, FVTh, pre = vstate.pop(b)
                if pre is None:
                    pre = _emit_araw_E(b, FVn, FVTh)
                E, ivs1 = pre

                FMT = FMTs[b % 2]
                for nv in range(2):
                    ps2 = tp.tile([1, 512], F32, tag="tp")
                    nc.tensor.matmul(
                        ps2, onesc[:NL, :], E[:NL, nv * 512 : (nv + 1) * 512],
                        start=True, stop=True,
                    )
                    nc.vector.tensor_copy(FMT[96:97, nv * 512 : (nv + 1) * 512], ps2)


                # E^T blocks + s2 (column sums of E)
                ET = smp.tile([P, 8, NL], F32R, tag="ET")
                s2 = smp.tile([P, 8], F32, tag="s2")
                for tg in range(2):
                    ps = tp.tile([P, 512], F32R, tag="tp")
                    for tt in range(4):
                        t = tg * 4 + tt
                        nc.tensor.transpose(
                            ps[:, tt * P : (tt + 1) * P],
                            E[:, t * P : (t + 1) * P],
                            iden,
                        )
                    psv = ps.rearrange("p (four c) -> p four c", four=4)[:, :, :NL]
                    nc.scalar.activation(ET[:, tg * 4 : (tg + 1) * 4, :], psv, AF.Copy)
                    nc.vector.reduce_sum(
                        s2[:, tg * 4 : (tg + 1) * 4],
                        ET[:, tg * 4 : (tg + 1) * 4, :],
                        axis=mybir.AxisListType.X,
                    )
                ivs2 = smp.tile([P, 8], F32, tag="ivs2")
                nc.vector.reciprocal(ivs2, s2)

                if pending_finals:
                    pending_finals.pop(0)()

                # h1 = E @ fv -> [77, 768]; scaled by 1/s1 on copy-back
                psh = accp.tile([NL, DV], F32, tag="acc")
                for c0, cw in ((0, 512), (512, 256)):
                    sl = psh[:, c0 : c0 + cw]
                    for t in range(8):
                        nc.tensor.matmul(
                            sl, ET[:, t, :], FVn[:, t, c0 : c0 + cw],
                            start=(t == 0), stop=(t == 7),
                        )
                h1n = smp.tile([P, DV], F32R, tag="h1n")
                nc.scalar.activation(h1n[:NL, :], psh, AF.Identity, scale=ivs1)

                # H1T = h1n^T : [768, 77]
                H1T = smp.tile([P, 6, NL + 1], F32R, tag="H1T")
                for kg in range(2):
                    ps = tp.tile([P, 384], F32R, tag="tp")
                    for kk in range(3):
                        ko = kg * 3 + kk
                        nc.tensor.transpose(
                            ps[:, kk * P : (kk + 1) * P],
                            h1n[:, ko * P : (ko + 1) * P],
                            iden,
                        )
                    psv = ps.rearrange("p (three c) -> p three c", three=3)[:, :, : NL + 1]
                    if kg == 0:
                        nc.vector.tensor_copy(H1T[:, kg * 3 : (kg + 1) * 3, :], psv)
                    else:
                        nc.scalar.activation(H1T[:, kg * 3 : (kg + 1) * 3, :], psv, AF.Copy)

                if b + 1 < BL:
                    _emit_load_tg0(b + 1)

                if pending_finals:
                    pending_finals.pop(0)()

                # fa_v^T = W_vv^T @ h1n^T + b_vv : [512, 77]
                FAVT = smp.tile([P, 4, NL + 1], F32R, tag="FAVT")
                for m in range(4):
                    ps = tp.tile([P, NL + 1], F32, tag="tp")
                    for ko in range(6):
                        nc.tensor.matmul(
                            ps, Wvv[:, ko, m * P : (m + 1) * P], H1T[:, ko, :],
                            start=(ko == 0), stop=(ko == 5),
                        )
                    nc.scalar.activation(
                        FAVT[:, m, :], ps, AF.Identity, bias=bvv[:, m, None]
                    )

                # m_small = fv_l @ fa_v^T : [77, 77]
                MS = smp.tile([NL, NL + 1], F32R, tag="MS")
                psm = tp.tile([NL, NL + 1], F32, tag="tp")
                for ko in range(4):
                    nc.tensor.matmul(
                        psm, FVL[:, ko, ls : ls + NL], FAVT[:, ko, :],
                        start=(ko == 0), stop=(ko == 3),
                    )
                nc.vector.tensor_copy(MS, psm)

                if pending_finals:
                    pending_finals.pop(0)()

                if b + 1 in vstate and len(vstate[b + 1][1]) == 1:
                    vstate[b + 1][1].append(_emit_tg(vstate[b + 1][0], 1))

                # fmT_un = m_small^T @ E : [77, 1024]; row 77 <- s2 row
                psf = accp.tile([NL, NV], F32, tag="acc")
                for nv in range(2):
                    nc.tensor.matmul(
                        psf[:, nv * 512 : (nv + 1) * 512],
                        MS[:, :NL], E[:NL, nv * 512 : (nv + 1) * 512],
                        start=True, stop=True,
                    )
                nc.vector.tensor_copy(FMT[:NL, :512], psf[:, :512])
                nc.scalar.activation(FMT[:NL, 512:], psf[:, 512:], AF.Copy)

                # finals for this batch are emitted during the NEXT batch
                # (software pipelining: their PE/copy/DMA work fills the
                # next batch's dependency stalls)
                def _emit_finals(b=b, FMT=FMT, ivs2=ivs2, ts=None):
                    for t in (ts if ts is not None else range(8)):
                        pso = accp.tile([P, OD], F32, tag="acc")
                        for c0, cw in ((0, 512), (512, 256)):
                            nc.tensor.matmul(
                                pso[:, c0 : c0 + cw],
                                FMT[:, t * P : (t + 1) * P],
                                Wm[:, c0 : c0 + cw],
                                start=True, stop=True,
                            )
                        OT = outp.tile([P, OD], F32, tag="OT")
                        if t % 2 == 0:
                            nc.vector.tensor_scalar_mul(OT, pso, ivs2[:, t, None])
                        else:
                            nc.scalar.activation(
                                OT, pso, AF.Identity, scale=ivs2[:, t, None]
                            )
                        nc.sync.dma_start(out_d[b, t * P : (t + 1) * P, :], OT)
                import functools as _ft
                pending_finals.append(_ft.partial(_emit_finals, ts=range(0, 3)))
                pending_finals.append(_ft.partial(_emit_finals, ts=range(3, 6)))
                pending_finals.append(_ft.partial(_emit_finals, ts=range(6, 8)))

                if b + 1 in vstate:
                    vstate[b + 1][2] = _emit_araw_E(
                        b + 1, vstate[b + 1][0], vstate[b + 1][1]
                    )

            for f in pending_finals:
                f()
            fvn3_pool.__exit__(None, None, None)

    nc.compile()
    return nc


_NC_CACHE = None
_last_in_maps = None


def _get_nc():
    global _NC_CACHE
    if _NC_CACHE is None:
        _NC_CACHE = _build()
    return _NC_CACHE


def kernel(**inputs) -> np.ndarray:
    fv = inputs["fv"]
    fl = inputs["fl"]
    consts = {
        "wlk": round_f32r(inputs["W_lk"]),
        "wlv": round_f32r(inputs["W_lv"]),
        "wvkT": round_f32r(np.ascontiguousarray(inputs["W_vk"].T)),
        "wvv": round_f32r(inputs["W_vv"]),
        "wm": round_f32r(
            np.concatenate(
                [
                    np.asarray(inputs["W_m"]),
                    np.zeros((19, OD), np.float32),
                    np.asarray(inputs["b_m"])[None, :],
                ],
                axis=0,
            )
        ),
        "blk": np.ascontiguousarray(inputs["b_lk"], dtype=np.float32),
        "blv": np.ascontiguousarray(inputs["b_lv"], dtype=np.float32),
        "bvv": np.ascontiguousarray(inputs["b_vv"], dtype=np.float32),
        "bvk": round_f32r(inputs["b_vk"]),
        "iden": np.eye(P, dtype=np.float32),
        "onesr": np.ones((1, 512), dtype=np.float32),
        "onesc": np.ones((P, 1), dtype=np.float32),
        "zeros": np.zeros((19, NV), dtype=np.float32),
    }
    fvr = round_f32r(fv)
    flr = round_f32r(fl)
    in_maps = []
    for c in range(NCORES):
        m = dict(consts)
        m["fv"] = np.ascontiguousarray(fvr[c * BL : (c + 1) * BL])
        m["fl"] = np.ascontiguousarray(flr[c * BL : (c + 1) * BL])
        in_maps.append(m)

    global _last_in_maps
    _last_in_maps = in_maps
    nc = _get_nc()
    res = run_bass_kernel_spmd(nc, in_maps, core_ids=list(range(NCORES)))
    out = np.concatenate([res.results[c]["out"] for c in range(NCORES)], axis=0)
    return np.ascontiguousarray(out, dtype=np.float32)


# revision 80
# speedup vs baseline: 1.0283x; 1.0283x over previous
"""Trainium2 Bass kernel for DenseLanguageGuidanceModule.

Math (per batch b):
    fk_l = fl @ W_lk + b_lk            [77, 512]
    fv-side projections are folded away algebraically:
      a_raw = (fk_l @ W_vk^T) @ fv^T + (fk_l @ b_vk) 1^T   (/= sqrt(512))
      fa_v  = diag(1/s1) (E @ fv) @ W_vv + b_vv,  E = exp(a_raw/sqrt(512))
      fm    = diag(1/s2) E^T @ (fv_l @ fa_v^T)
      out   = fm @ W_m + b_m
    where s1 = row sums of E, s2 = column sums of E.

Distribution: pure data-parallel over batch B=32 across 8 NeuronCores
(4 batches per core), weights replicated. No collectives.

All matmuls run in float32r (TF32-like: 11 mantissa bits, fp32 accumulate)
which is full PE speed for free-dim >= 256. Inputs are pre-rounded to f32r
on the host (RNE, keep top 20 bits) so on-device rounding is a no-op.
"""
import sys

sys.path.insert(0, "/opt/trn_rl_repo")

import numpy as np

import concourse.bass as bass  # noqa: E402
import concourse.tile as tile  # noqa: E402
from concourse import bacc, mybir  # noqa: E402
from concourse.bass_utils import run_bass_kernel_spmd  # noqa: E402

P = 128
NCORES = 8
B = 32
BL = B // NCORES          # 4 batches per core
NV, DV = 1024, 768        # vision tokens / dim
NL, DL = 77, 512          # language tokens / dim
D = 512                   # shared feature dim
OD = 768                  # output dim
NLB = NL * BL             # 308: l-dim stacked across local batches

F32R = mybir.dt.float32r
F32 = mybir.dt.float32
ISQD = 1.0 / float(np.sqrt(np.float32(D)))

AF = mybir.ActivationFunctionType


def round_f32r(x: np.ndarray) -> np.ndarray:
    """RNE-round fp32 to f32r (1s+8e+11m in the top 20 bits)."""
    u = np.ascontiguousarray(x, dtype=np.float32).view(np.uint32)
    low = u & np.uint32(0xFFF)
    base = u & np.uint32(0xFFFFF000)
    lsb = (u >> np.uint32(12)) & np.uint32(1)
    up = (low > 0x800) | ((low == 0x800) & (lsb == 1))
    return (base + np.where(up, np.uint32(0x1000), np.uint32(0))).view(np.float32)


def _build():
    nc = bacc.Bacc("TRN2", target_bir_lowering=False)

    fv_d = nc.dram_tensor("fv", [BL, NV, DV], F32R, kind="ExternalInput")
    fl_d = nc.dram_tensor("fl", [BL, NL, DL], F32R, kind="ExternalInput")
    wkc_d = nc.dram_tensor("wkc", [DL, DV], F32R, kind="ExternalInput")
    wlv_d = nc.dram_tensor("wlv", [DL, D], F32R, kind="ExternalInput")
    wvkT_d = nc.dram_tensor("wvkT", [D, DV], F32R, kind="ExternalInput")
    wvv_d = nc.dram_tensor("wvv", [DV, D], F32R, kind="ExternalInput")
    wm_d = nc.dram_tensor("wm", [97, OD], F32R, kind="ExternalInput")
    blv_d = nc.dram_tensor("blv", [DL], F32, kind="ExternalInput")
    bvv_d = nc.dram_tensor("bvv", [D], F32, kind="ExternalInput")
    wc_d = nc.dram_tensor("wc", [DL], F32R, kind="ExternalInput")
    c2_d = nc.dram_tensor("c2", [DV], F32, kind="ExternalInput")
    cc_d = nc.dram_tensor("cc", [1], F32, kind="ExternalInput")
    iden_d = nc.dram_tensor("iden", [P, P], F32R, kind="ExternalInput")
    onesr_d = nc.dram_tensor("onesr", [1, 512], F32R, kind="ExternalInput")
    onesc_d = nc.dram_tensor("onesc", [P, 1], F32R, kind="ExternalInput")
    zeros_d = nc.dram_tensor("zeros", [19, NV], F32R, kind="ExternalInput")
    out_d = nc.dram_tensor("out", [BL, NV, OD], F32, kind="ExternalOutput")

    with tile.TileContext(nc) as tc:
        with (
            tc.tile_pool(name="consts", bufs=1) as cp,
            tc.tile_pool(name="lph", bufs=1) as lp,
            tc.tile_pool(name="fvn", bufs=2) as fvnp,
            tc.tile_pool(name="fvt", bufs=3) as fvtp,
            tc.tile_pool(name="eb", bufs=2) as ebp,
            tc.tile_pool(name="sm", bufs=2) as smp,
            tc.tile_pool(name="outp", bufs=4) as outp,
            tc.tile_pool(name="tp", bufs=4, space="PSUM") as tp,       # 1-bank slots
            tc.tile_pool(name="acc", bufs=2, space="PSUM") as accp,    # 2-bank slots
        ):
            # ---------------- constants (early: identity only) ----------------
            iden = cp.tile([P, P], F32R)
            nc.sync.dma_start(iden, iden_d[:, :])
            # ---------------- language phase (batched over BL) ----------------
            lph_tmp = tc.tile_pool(name="lphtmp", bufs=1)
            lpt = lph_tmp.__enter__()
            # FLT = fl_all^T  [512(D1 on p), 308]
            FLT = lpt.tile([P, 4, NLB], F32R)
            with tc.tile_pool(name="fln", bufs=1) as flnp:
                fl_flat = fl_d.rearrange("b l d -> (b l) d")
                row_tiles = [(0, P), (P, P), (2 * P, NLB - 2 * P)]
                for half in range(2):
                    c0 = half * 256
                    FLn = flnp.tile([P, 3, 256], F32R, tag="fln")
                    for i, (r0, sz) in enumerate(row_tiles):
                        nc.sync.dma_start(
                            FLn[:sz, i, :], fl_flat[r0 : r0 + sz, c0 : c0 + 256]
                        )
                    for fb in range(2):
                        ps = tp.tile([P, 384], F32R, tag="tp")
                        for i in range(3):
                            nc.tensor.transpose(
                                ps[:, i * P : (i + 1) * P],
                                FLn[:, i, fb * P : (fb + 1) * P],
                                iden,
                            )
                        fbo = half * 2 + fb
                        nc.vector.tensor_copy(FLT[:, fbo, : 2 * P], ps[:, : 2 * P])
                        nc.vector.tensor_copy(
                            FLT[:, fbo, 2 * P :], ps[:, 2 * P : 2 * P + (NLB - 2 * P)]
                        )

            # ---------------- remaining constants, interleaved with fv[0] ----------------
            FVn0 = fvnp.tile([P, 8, DV], F32R, tag="fvn")
            fvb0 = fv_d[0].rearrange("(t p) d -> p t d", p=P)
            Wkc = lpt.tile([P, 4, DV], F32R)
            nc.sync.dma_start(Wkc, wkc_d.rearrange("(ko p) m -> p ko m", p=P))
            c2t = cp.tile([P, 6], F32)
            nc.sync.dma_start(c2t, c2_d.rearrange("(ko p) -> p ko", p=P))
            Wlv = lpt.tile([P, 4, D], F32R)
            nc.sync.dma_start(Wlv, wlv_d.rearrange("(ko p) m -> p ko m", p=P))
            blv = cp.tile([P, 4], F32)
            nc.sync.dma_start(blv, blv_d.rearrange("(ko p) -> p ko", p=P))

            wct = cp.tile([P, 4], F32R)
            nc.sync.dma_start(wct, wc_d.rearrange("(ko p) -> p ko", p=P))
            cct = cp.tile([1, 1], F32)
            nc.sync.dma_start(cct, cc_d[None, :])
            for q in range(4):
                nc.sync.dma_start(FVn0[:, 2 * q : 2 * q + 2, :], fvb0[:, 2 * q : 2 * q + 2, :])
            onesr = cp.tile([1, 512], F32R)
            nc.sync.dma_start(onesr, onesr_d[:, :])
            onesc = cp.tile([P, 1], F32R)
            nc.sync.dma_start(onesc, onesc_d[:, :])


            # FVL = fv_l^T (+b_lv): [512, 308]
            FVL = lp.tile([P, 4, NLB], F32R)
            for m in range(4):
                ps = tp.tile([P, NLB], F32, tag="tp")
                for ko in range(4):
                    nc.tensor.matmul(
                        ps, Wlv[:, ko, m * P : (m + 1) * P], FLT[:, ko, :],
                        start=(ko == 0), stop=(ko == 3),
                    )
                nc.vector.tensor_scalar_add(FVL[:, m, :], ps, blv[:, m, None])

            # GT = g^T = (W_lk @ W_vk^T)^T @ fl^T + c2 : [768, 308]
            GT = lp.tile([P, 6, NLB], F32R)
            for mv in range(6):
                ps = tp.tile([P, NLB], F32, tag="tp")
                for ko in range(4):
                    nc.tensor.matmul(
                        ps, Wkc[:, ko, mv * P : (mv + 1) * P], FLT[:, ko, :],
                        start=(ko == 0), stop=(ko == 3),
                    )
                if mv % 2 == 0:
                    nc.vector.tensor_scalar_add(GT[:, mv, :], ps, c2t[:, mv, None])
                else:
                    nc.scalar.activation(
                        GT[:, mv, :], ps, AF.Identity, bias=c2t[:, mv, None]
                    )

            # C = (fk_l @ b_vk)^T : [1, 308]
            Cst = lp.tile([1, NLB], F32R)
            psc = tp.tile([1, NLB], F32, tag="tp")
            for ko in range(4):
                nc.tensor.matmul(
                    psc, wct[:, ko, None], FLT[:, ko, :],
                    start=(ko == 0), stop=(ko == 3),
                )
            nc.vector.tensor_scalar_add(Cst, psc, cct[:, :])
            lph_tmp.__exit__(None, None, None)
            fvn3_pool = tc.tile_pool(name="fvn3", bufs=1)
            fvn3 = fvn3_pool.__enter__()

            # late-use constants (not needed until fa_v / final)
            Wvv = cp.tile([P, 6, D], F32R)
            nc.sync.dma_start(Wvv, wvv_d.rearrange("(ko p) m -> p ko m", p=P))
            Wm = cp.tile([97, OD], F32R)
            nc.sync.dma_start(Wm, wm_d[:, :])
            bvv = cp.tile([P, 4], F32)
            nc.sync.dma_start(bvv, bvv_d.rearrange("(ko p) -> p ko", p=P))


            # persistent FMT ping-pong pair; filler rows 77..95 zeroed once
            FMTs = []
            for _i in range(2):
                _f = lp.tile([97, NV], F32R, tag=f"FMT{_i}")
                nc.sync.dma_start(_f[NL:96, :], zeros_d[:, :])
                FMTs.append(_f)

            # ---------------- per-batch vision phase ----------------
            pending_finals = []
            vstate = {}

            def _emit_tg(FVn, tg):
                fvth = fvtp.tile([P, 6, 512], F32R, tag="fvt")
                for ko in range(6):
                    ps = tp.tile([P, 512], F32R, tag="tp")
                    for tt in range(4):
                        t = tg * 4 + tt
                        nc.tensor.transpose(
                            ps[:, tt * P : (tt + 1) * P],
                            FVn[:, t, ko * P : (ko + 1) * P],
                            iden,
                        )
                    if (ko + tg) % 2 == 0:
                        nc.vector.tensor_copy(fvth[:, ko, :], ps)
                    else:
                        nc.scalar.activation(fvth[:, ko, :], ps, AF.Copy)
                return fvth

            def _emit_load_tg0(nb):
                if nb == 0:
                    FVn = FVn0
                else:
                    pool_b = fvn3 if nb == 2 else fvnp
                    FVn = pool_b.tile([P, 8, DV], F32R, tag="fvn")
                    fvb = fv_d[nb].rearrange("(t p) d -> p t d", p=P)
                    for q in range(4):
                        nc.sync.dma_start(
                            FVn[:, 2 * q : 2 * q + 2, :], fvb[:, 2 * q : 2 * q + 2, :]
                        )
                vstate[nb] = [FVn, [_emit_tg(FVn, 0)], None]

            def _emit_araw_E(nb, FVn, FVTh):
                nls = nb * NL
                if len(FVTh) == 1:
                    FVTh.append(_emit_tg(FVn, 1))
                # a_raw = g @ fv^T + c 1^T  -> psum [77, 1024]
                psa = accp.tile([NL, NV], F32, tag="acc")
                for nv in range(2):
                    sl = psa[:, nv * 512 : (nv + 1) * 512]
                    for ko in range(6):
                        nc.tensor.matmul(
                            sl, GT[:, ko, nls : nls + NL],
                            FVTh[nv][:, ko, :],
                            start=(ko == 0), stop=False,
                        )
                    nc.tensor.matmul(
                        sl, Cst[:1, nls : nls + NL],
                        onesr[:1, :512],
                        start=False, stop=True,
                    )
                # E = exp(a/sqrt(D)); s1 = row sums
                E = ebp.tile([P, NV], F32R, tag="E")
                s1p = smp.tile([NL, 2], F32, tag="s1p")
                for nv in range(2):
                    nc.scalar.activation(
                        E[:NL, nv * 512 : (nv + 1) * 512],
                        psa[:, nv * 512 : (nv + 1) * 512],
                        AF.Exp, scale=ISQD, accum_out=s1p[:, nv, None],
                    )
                s1 = smp.tile([NL, 1], F32, tag="s1")
                nc.vector.reduce_sum(s1, s1p, axis=mybir.AxisListType.X)
                ivs1 = smp.tile([NL, 1], F32, tag="ivs1")
                nc.vector.reciprocal(ivs1, s1)
                return (E, ivs1)

            _emit_load_tg0(0)
            for b in range(BL):
                ls = b * NL  # column offset of this batch in *_all tensors

                FVn, FVTh, pre = vstate.pop(b)
                if pre is None:
                    pre = _emit_araw_E(b, FVn, FVTh)
                E, ivs1 = pre

                FMT = FMTs[b % 2]
                for nv in range(2):
                    ps2 = tp.tile([1, 512], F32, tag="tp")
                    nc.tensor.matmul(
                        ps2, onesc[:NL, :], E[:NL, nv * 512 : (nv + 1) * 512],
                        start=True, stop=True,
                    )
                    nc.vector.tensor_copy(FMT[96:97, nv * 512 : (nv + 1) * 512], ps2)


                # E^T blocks + s2 (column sums of E)
                ET = smp.tile([P, 8, NL], F32R, tag="ET")
                s2 = smp.tile([P, 8], F32, tag="s2")
                for tg in range(2):
                    ps = tp.tile([P, 512], F32R, tag="tp")
                    for tt in range(4):
                        t = tg * 4 + tt
                        nc.tensor.transpose(
                            ps[:, tt * P : (tt + 1) * P],
                            E[:, t * P : (t + 1) * P],
                            iden,
                        )
                    psv = ps.rearrange("p (four c) -> p four c", four=4)[:, :, :NL]
                    nc.scalar.activation(ET[:, tg * 4 : (tg + 1) * 4, :], psv, AF.Copy)
                    nc.vector.reduce_sum(
                        s2[:, tg * 4 : (tg + 1) * 4],
                        ET[:, tg * 4 : (tg + 1) * 4, :],
                        axis=mybir.AxisListType.X,
                    )
                ivs2 = smp.tile([P, 8], F32, tag="ivs2")
                nc.vector.reciprocal(ivs2, s2)

                if pending_finals:
                    pending_finals.pop(0)()

                # h1 = E @ fv -> [77, 768]; scaled by 1/s1 on copy-back
                psh = accp.tile([NL, DV], F32, tag="acc")
                for c0, cw in ((0, 512), (512, 256)):
                    sl = psh[:, c0 : c0 + cw]
                    for t in range(8):
                        nc.tensor.matmul(
                            sl, ET[:, t, :], FVn[:, t, c0 : c0 + cw],
                            start=(t == 0), stop=(t == 7),
                        )
                h1n = smp.tile([P, DV], F32R, tag="h1n")
                nc.scalar.activation(h1n[:NL, :], psh, AF.Identity, scale=ivs1)

                # H1T = h1n^T : [768, 77]
                H1T = smp.tile([P, 6, NL + 1], F32R, tag="H1T")
                for kg in range(2):
                    ps = tp.tile([P, 384], F32R, tag="tp")
                    for kk in range(3):
                        ko = kg * 3 + kk
                        nc.tensor.transpose(
                            ps[:, kk * P : (kk + 1) * P],
                            h1n[:, ko * P : (ko + 1) * P],
                            iden,
                        )
                    psv = ps.rearrange("p (three c) -> p three c", three=3)[:, :, : NL + 1]
                    if kg == 0:
                        nc.vector.tensor_copy(H1T[:, kg * 3 : (kg + 1) * 3, :], psv)
                    else:
                        nc.scalar.activation(H1T[:, kg * 3 : (kg + 1) * 3, :], psv, AF.Copy)

                if b + 1 < BL:
                    _emit_load_tg0(b + 1)

                if pending_finals:
                    pending_finals.pop(0)()

                # fa_v^T = W_vv^T @ h1n^T + b_vv : [512, 77]
                FAVT = smp.tile([P, 4, NL + 1], F32R, tag="FAVT")
                for m in range(4):
                    ps = tp.tile([P, NL + 1], F32, tag="tp")
                    for ko in range(6):
                        nc.tensor.matmul(
                            ps, Wvv[:, ko, m * P : (m + 1) * P], H1T[:, ko, :],
                            start=(ko == 0), stop=(ko == 5),
                        )
                    nc.scalar.activation(
                        FAVT[:, m, :], ps, AF.Identity, bias=bvv[:, m, None]
                    )

                # m_small = fv_l @ fa_v^T : [77, 77]
                MS = smp.tile([NL, NL + 1], F32R, tag="MS")
                psm = tp.tile([NL, NL + 1], F32, tag="tp")
                for ko in range(4):
                    nc.tensor.matmul(
                        psm, FVL[:, ko, ls : ls + NL], FAVT[:, ko, :],
                        start=(ko == 0), stop=(ko == 3),
                    )
                nc.vector.tensor_copy(MS, psm)

                if pending_finals:
                    pending_finals.pop(0)()

                if b + 1 in vstate and len(vstate[b + 1][1]) == 1:
                    vstate[b + 1][1].append(_emit_tg(vstate[b + 1][0], 1))

                # fmT_un = m_small^T @ E : [77, 1024]; row 77 <- s2 row
                psf = accp.tile([NL, NV], F32, tag="acc")
                for nv in range(2):
                    nc.tensor.matmul(
                        psf[:, nv * 512 : (nv + 1) * 512],
                        MS[:, :NL], E[:NL, nv * 512 : (nv + 1) * 512],
                        start=True, stop=True,
                    )
                nc.vector.tensor_copy(FMT[:NL, :512], psf[:, :512])
                nc.scalar.activation(FMT[:NL, 512:], psf[:, 512:], AF.Copy)

                # finals for this batch are emitted during the NEXT batch
                # (software pipelining: their PE/copy/DMA work fills the
                # next batch's dependency stalls)
                def _emit_finals(b=b, FMT=FMT, ivs2=ivs2, ts=None):
                    for t in (ts if ts is not None else range(8)):
                        pso = accp.tile([P, OD], F32, tag="acc")
                        for c0, cw in ((0, 512), (512, 256)):
                            nc.tensor.matmul(
                                pso[:, c0 : c0 + cw],
                                FMT[:, t * P : (t + 1) * P],
                                Wm[:, c0 : c0 + cw],
                                start=True, stop=True,
                            )
                        OT = outp.tile([P, OD], F32, tag="OT")
                        if t % 2 == 0:
                            nc.vector.tensor_scalar_mul(OT, pso, ivs2[:, t, None])
                        else:
                            nc.scalar.activation(
                                OT, pso, AF.Identity, scale=ivs2[:, t, None]
                            )
                        nc.sync.dma_start(out_d[b, t * P : (t + 1) * P, :], OT)
                import functools as _ft
                pending_finals.append(_ft.partial(_emit_finals, ts=range(0, 3)))
                pending_finals.append(_ft.partial(_emit_finals, ts=range(3, 6)))
                pending_finals.append(_ft.partial(_emit_finals, ts=range(6, 8)))

                if b + 1 in vstate:
                    vstate[b + 1][2] = _emit_araw_E(
                        b + 1, vstate[b + 1][0], vstate[b + 1][1]
                    )

            for f in pending_finals:
                f()
            fvn3_pool.__exit__(None, None, None)

    nc.compile()
    return nc


_NC_CACHE = None
_last_in_maps = None


def _get_nc():
    global _NC_CACHE
    if _NC_CACHE is None:
        _NC_CACHE = _build()
    return _NC_CACHE


def kernel(**inputs) -> np.ndarray:
    fv = inputs["fv"]
    fl = inputs["fl"]
    consts = {
        "wkc": round_f32r(np.asarray(inputs["W_lk"]) @ np.asarray(inputs["W_vk"]).T),
        "wlv": round_f32r(inputs["W_lv"]),
        "wvkT": round_f32r(np.ascontiguousarray(inputs["W_vk"].T)),
        "wvv": round_f32r(inputs["W_vv"]),
        "wm": round_f32r(
            np.concatenate(
                [
                    np.asarray(inputs["W_m"]),
                    np.zeros((19, OD), np.float32),
                    np.asarray(inputs["b_m"])[None, :],
                ],
                axis=0,
            )
        ),
        "blv": np.ascontiguousarray(inputs["b_lv"], dtype=np.float32),
        "bvv": np.ascontiguousarray(inputs["b_vv"], dtype=np.float32),
        "wc": round_f32r(np.asarray(inputs["W_lk"]) @ np.asarray(inputs["b_vk"])),
        "cc": np.array(
            [float(np.asarray(inputs["b_lk"]) @ np.asarray(inputs["b_vk"]))],
            dtype=np.float32,
        ),
        "c2": np.ascontiguousarray(
            np.asarray(inputs["W_vk"]) @ np.asarray(inputs["b_lk"]), dtype=np.float32
        ),
        "iden": np.eye(P, dtype=np.float32),
        "onesr": np.ones((1, 512), dtype=np.float32),
        "onesc": np.ones((P, 1), dtype=np.float32),
        "zeros": np.zeros((19, NV), dtype=np.float32),
    }
    fvr = round_f32r(fv)
    flr = round_f32r(fl)
    in_maps = []
    for c in range(NCORES):
        m = dict(consts)
        m["fv"] = np.ascontiguousarray(fvr[c * BL : (c + 1) * BL])
        m["fl"] = np.ascontiguousarray(flr[c * BL : (c + 1) * BL])
        in_maps.append(m)

    global _last_in_maps
    _last_in_maps = in_maps
    nc = _get_nc()
    res = run_bass_kernel_spmd(nc, in_maps, core_ids=list(range(NCORES)))
    out = np.concatenate([res.results[c]["out"] for c in range(NCORES)], axis=0)
    return np.ascontiguousarray(out, dtype=np.float32)


# revision 83
# speedup vs baseline: 1.0342x; 1.0057x over previous
"""Trainium2 Bass kernel for DenseLanguageGuidanceModule.

Math (per batch b):
    fk_l = fl @ W_lk + b_lk            [77, 512]
    fv-side projections are folded away algebraically:
      a_raw = (fk_l @ W_vk^T) @ fv^T + (fk_l @ b_vk) 1^T   (/= sqrt(512))
      fa_v  = diag(1/s1) (E @ fv) @ W_vv + b_vv,  E = exp(a_raw/sqrt(512))
      fm    = diag(1/s2) E^T @ (fv_l @ fa_v^T)
      out   = fm @ W_m + b_m
    where s1 = row sums of E, s2 = column sums of E.

Distribution: pure data-parallel over batch B=32 across 8 NeuronCores
(4 batches per core), weights replicated. No collectives.

All matmuls run in float32r (TF32-like: 11 mantissa bits, fp32 accumulate)
which is full PE speed for free-dim >= 256. Inputs are pre-rounded to f32r
on the host (RNE, keep top 20 bits) so on-device rounding is a no-op.
"""
import sys

sys.path.insert(0, "/opt/trn_rl_repo")

import numpy as np

import concourse.bass as bass  # noqa: E402
import concourse.tile as tile  # noqa: E402
from concourse import bacc, mybir  # noqa: E402
from concourse.bass_utils import run_bass_kernel_spmd  # noqa: E402

P = 128
NCORES = 8
B = 32
BL = B // NCORES          # 4 batches per core
NV, DV = 1024, 768        # vision tokens / dim
NL, DL = 77, 512          # language tokens / dim
D = 512                   # shared feature dim
OD = 768                  # output dim
NLB = NL * BL             # 308: l-dim stacked across local batches

F32R = mybir.dt.float32r
F32 = mybir.dt.float32
ISQD = 1.0 / float(np.sqrt(np.float32(D)))

AF = mybir.ActivationFunctionType


def round_f32r(x: np.ndarray) -> np.ndarray:
    """RNE-round fp32 to f32r (1s+8e+11m in the top 20 bits)."""
    u = np.ascontiguousarray(x, dtype=np.float32).view(np.uint32)
    low = u & np.uint32(0xFFF)
    base = u & np.uint32(0xFFFFF000)
    lsb = (u >> np.uint32(12)) & np.uint32(1)
    up = (low > 0x800) | ((low == 0x800) & (lsb == 1))
    return (base + np.where(up, np.uint32(0x1000), np.uint32(0))).view(np.float32)


def _build():
    nc = bacc.Bacc("TRN2", target_bir_lowering=False)

    fv_d = nc.dram_tensor("fv", [BL, NV, DV], F32R, kind="ExternalInput")
    fl_d = nc.dram_tensor("fl", [BL, NL, DL], F32R, kind="ExternalInput")
    wkc_d = nc.dram_tensor("wkc", [DL, DV], F32R, kind="ExternalInput")
    wvc_d = nc.dram_tensor("wvc", [DL, DV], F32R, kind="ExternalInput")
    wvkT_d = nc.dram_tensor("wvkT", [D, DV], F32R, kind="ExternalInput")
    wm_d = nc.dram_tensor("wm", [97, OD], F32R, kind="ExternalInput")
    wc_d = nc.dram_tensor("wc", [DL], F32R, kind="ExternalInput")
    c2_d = nc.dram_tensor("c2", [DV], F32, kind="ExternalInput")
    cc_d = nc.dram_tensor("cc", [1], F32, kind="ExternalInput")
    c2v_d = nc.dram_tensor("c2v", [DV], F32, kind="ExternalInput")
    wcv_d = nc.dram_tensor("wcv", [DL], F32R, kind="ExternalInput")
    ccv_d = nc.dram_tensor("ccv", [1], F32, kind="ExternalInput")
    iden_d = nc.dram_tensor("iden", [P, P], F32R, kind="ExternalInput")
    onesr_d = nc.dram_tensor("onesr", [1, 512], F32R, kind="ExternalInput")
    onesc_d = nc.dram_tensor("onesc", [P, 1], F32R, kind="ExternalInput")
    zeros_d = nc.dram_tensor("zeros", [19, NV], F32R, kind="ExternalInput")
    out_d = nc.dram_tensor("out", [BL, NV, OD], F32, kind="ExternalOutput")

    with tile.TileContext(nc) as tc:
        with (
            tc.tile_pool(name="consts", bufs=1) as cp,
            tc.tile_pool(name="lph", bufs=1) as lp,
            tc.tile_pool(name="fvn", bufs=2) as fvnp,
            tc.tile_pool(name="fvt", bufs=3) as fvtp,
            tc.tile_pool(name="eb", bufs=2) as ebp,
            tc.tile_pool(name="sm", bufs=2) as smp,
            tc.tile_pool(name="outp", bufs=4) as outp,
            tc.tile_pool(name="tp", bufs=4, space="PSUM") as tp,       # 1-bank slots
            tc.tile_pool(name="acc", bufs=2, space="PSUM") as accp,    # 2-bank slots
        ):
            # ---------------- constants (early: identity only) ----------------
            iden = cp.tile([P, P], F32R)
            nc.sync.dma_start(iden, iden_d[:, :])
            # ---------------- language phase (batched over BL) ----------------
            lph_tmp = tc.tile_pool(name="lphtmp", bufs=1)
            lpt = lph_tmp.__enter__()
            # FLT = fl_all^T  [512(D1 on p), 308]
            FLT = lpt.tile([P, 4, NLB], F32R)
            with tc.tile_pool(name="fln", bufs=1) as flnp:
                fl_flat = fl_d.rearrange("b l d -> (b l) d")
                row_tiles = [(0, P), (P, P), (2 * P, NLB - 2 * P)]
                for half in range(2):
                    c0 = half * 256
                    FLn = flnp.tile([P, 3, 256], F32R, tag="fln")
                    for i, (r0, sz) in enumerate(row_tiles):
                        nc.sync.dma_start(
                            FLn[:sz, i, :], fl_flat[r0 : r0 + sz, c0 : c0 + 256]
                        )
                    for fb in range(2):
                        ps = tp.tile([P, 384], F32R, tag="tp")
                        for i in range(3):
                            nc.tensor.transpose(
                                ps[:, i * P : (i + 1) * P],
                                FLn[:, i, fb * P : (fb + 1) * P],
                                iden,
                            )
                        fbo = half * 2 + fb
                        nc.vector.tensor_copy(FLT[:, fbo, : 2 * P], ps[:, : 2 * P])
                        nc.vector.tensor_copy(
                            FLT[:, fbo, 2 * P :], ps[:, 2 * P : 2 * P + (NLB - 2 * P)]
                        )

            # ---------------- remaining constants, interleaved with fv[0] ----------------
            FVn0 = fvnp.tile([P, 8, DV], F32R, tag="fvn")
            fvb0 = fv_d[0].rearrange("(t p) d -> p t d", p=P)
            Wkc = lpt.tile([P, 4, DV], F32R)
            nc.sync.dma_start(Wkc, wkc_d.rearrange("(ko p) m -> p ko m", p=P))
            c2t = cp.tile([P, 6], F32)
            nc.sync.dma_start(c2t, c2_d.rearrange("(ko p) -> p ko", p=P))
            Wvc = lpt.tile([P, 4, DV], F32R)
            nc.sync.dma_start(Wvc, wvc_d.rearrange("(ko p) m -> p ko m", p=P))
            c2vt = cp.tile([P, 6], F32)
            nc.sync.dma_start(c2vt, c2v_d.rearrange("(ko p) -> p ko", p=P))
            wcvt = cp.tile([P, 4], F32R)
            nc.sync.dma_start(wcvt, wcv_d.rearrange("(ko p) -> p ko", p=P))
            ccvt = cp.tile([1, 1], F32)
            nc.sync.dma_start(ccvt, ccv_d[None, :])

            wct = cp.tile([P, 4], F32R)
            nc.sync.dma_start(wct, wc_d.rearrange("(ko p) -> p ko", p=P))
            cct = cp.tile([1, 1], F32)
            nc.sync.dma_start(cct, cc_d[None, :])
            for q in range(4):
                nc.sync.dma_start(FVn0[:, 2 * q : 2 * q + 2, :], fvb0[:, 2 * q : 2 * q + 2, :])
            onesr = cp.tile([1, 512], F32R)
            nc.sync.dma_start(onesr, onesr_d[:, :])
            onesc = cp.tile([P, 1], F32R)
            nc.sync.dma_start(onesc, onesc_d[:, :])


            # FWVT = (fv_l @ W_vv^T)^T = (W_lv@W_vv^T)^T @ fl^T + c2v : [768, 308]
            FWVT = lp.tile([P, 6, NLB], F32R)
            for mv in range(6):
                ps = tp.tile([P, NLB], F32, tag="tp")
                for ko in range(4):
                    nc.tensor.matmul(
                        ps, Wvc[:, ko, mv * P : (mv + 1) * P], FLT[:, ko, :],
                        start=(ko == 0), stop=(ko == 3),
                    )
                if mv % 2 == 0:
                    nc.vector.tensor_scalar_add(FWVT[:, mv, :], ps, c2vt[:, mv, None])
                else:
                    nc.scalar.activation(
                        FWVT[:, mv, :], ps, AF.Identity, bias=c2vt[:, mv, None]
                    )

            # Cv = (fv_l @ b_vv)^T : [1, 308]
            Cv = lp.tile([1, NLB], F32R)
            pscv = tp.tile([1, NLB], F32, tag="tp")
            for ko in range(4):
                nc.tensor.matmul(
                    pscv, wcvt[:, ko, None], FLT[:, ko, :],
                    start=(ko == 0), stop=(ko == 3),
                )
            nc.vector.tensor_scalar_add(Cv, pscv, ccvt[:, :])

            # GT = g^T = (W_lk @ W_vk^T)^T @ fl^T + c2 : [768, 308]
            GT = lp.tile([P, 6, NLB], F32R)
            for mv in range(6):
                ps = tp.tile([P, NLB], F32, tag="tp")
                for ko in range(4):
                    nc.tensor.matmul(
                        ps, Wkc[:, ko, mv * P : (mv + 1) * P], FLT[:, ko, :],
                        start=(ko == 0), stop=(ko == 3),
                    )
                if mv % 2 == 0:
                    nc.vector.tensor_scalar_add(GT[:, mv, :], ps, c2t[:, mv, None])
                else:
                    nc.scalar.activation(
                        GT[:, mv, :], ps, AF.Identity, bias=c2t[:, mv, None]
                    )

            # C = (fk_l @ b_vk)^T : [1, 308]
            Cst = lp.tile([1, NLB], F32R)
            psc = tp.tile([1, NLB], F32, tag="tp")
            for ko in range(4):
                nc.tensor.matmul(
                    psc, wct[:, ko, None], FLT[:, ko, :],
                    start=(ko == 0), stop=(ko == 3),
                )
            nc.vector.tensor_scalar_add(Cst, psc, cct[:, :])
            lph_tmp.__exit__(None, None, None)
            fvn3_pool = tc.tile_pool(name="fvn3", bufs=1)
            fvn3 = fvn3_pool.__enter__()

            # late-use constants (not needed until final)
            Wm = cp.tile([97, OD], F32R)
            nc.sync.dma_start(Wm, wm_d[:, :])


            # persistent FMT ping-pong pair; filler rows 77..95 zeroed once
            FMTs = []
            for _i in range(2):
                _f = lp.tile([97, NV], F32R, tag=f"FMT{_i}")
                nc.sync.dma_start(_f[NL:96, :], zeros_d[:, :])
                FMTs.append(_f)

            # ---------------- per-batch vision phase ----------------
            pending_finals = []
            vstate = {}

            def _emit_tg(FVn, tg):
                fvth = fvtp.tile([P, 6, 512], F32R, tag="fvt")
                for ko in range(6):
                    ps = tp.tile([P, 512], F32R, tag="tp")
                    for tt in range(4):
                        t = tg * 4 + tt
                        nc.tensor.transpose(
                            ps[:, tt * P : (tt + 1) * P],
                            FVn[:, t, ko * P : (ko + 1) * P],
                            iden,
                        )
                    if (ko + tg) % 2 == 0:
                        nc.vector.tensor_copy(fvth[:, ko, :], ps)
                    else:
                        nc.scalar.activation(fvth[:, ko, :], ps, AF.Copy)
                return fvth

            def _emit_load_tg0(nb):
                if nb == 0:
                    FVn = FVn0
                else:
                    pool_b = fvn3 if nb == 2 else fvnp
                    FVn = pool_b.tile([P, 8, DV], F32R, tag="fvn")
                    fvb = fv_d[nb].rearrange("(t p) d -> p t d", p=P)
                    for q in range(4):
                        nc.sync.dma_start(
                            FVn[:, 2 * q : 2 * q + 2, :], fvb[:, 2 * q : 2 * q + 2, :]
                        )
                vstate[nb] = [FVn, [_emit_tg(FVn, 0)], None]

            def _emit_araw_E(nb, FVn, FVTh):
                nls = nb * NL
                if len(FVTh) == 1:
                    FVTh.append(_emit_tg(FVn, 1))
                # a_raw = g @ fv^T + c 1^T  -> psum [77, 1024]
                psa = accp.tile([NL, NV], F32, tag="acc")
                for nv in range(2):
                    sl = psa[:, nv * 512 : (nv + 1) * 512]
                    for ko in range(6):
                        nc.tensor.matmul(
                            sl, GT[:, ko, nls : nls + NL],
                            FVTh[nv][:, ko, :],
                            start=(ko == 0), stop=False,
                        )
                    nc.tensor.matmul(
                        sl, Cst[:1, nls : nls + NL],
                        onesr[:1, :512],
                        start=False, stop=True,
                    )
                # E = exp(a/sqrt(D)); s1 = row sums
                E = ebp.tile([P, NV], F32R, tag="E")
                s1p = smp.tile([NL, 2], F32, tag="s1p")
                for nv in range(2):
                    nc.scalar.activation(
                        E[:NL, nv * 512 : (nv + 1) * 512],
                        psa[:, nv * 512 : (nv + 1) * 512],
                        AF.Exp, scale=ISQD, accum_out=s1p[:, nv, None],
                    )
                s1 = smp.tile([NL, 1], F32, tag="s1")
                nc.vector.reduce_sum(s1, s1p, axis=mybir.AxisListType.X)
                ivs1 = smp.tile([NL, 1], F32, tag="ivs1")
                nc.vector.reciprocal(ivs1, s1)
                return (E, ivs1)

            _emit_load_tg0(0)
            for b in range(BL):
                ls = b * NL  # column offset of this batch in *_all tensors

                FVn, FVTh, pre = vstate.pop(b)
                if pre is None:
                    pre = _emit_araw_E(b, FVn, FVTh)
                E, ivs1 = pre

                FMT = FMTs[b % 2]
                for nv in range(2):
                    ps2 = tp.tile([1, 512], F32, tag="tp")
                    nc.tensor.matmul(
                        ps2, onesc[:NL, :], E[:NL, nv * 512 : (nv + 1) * 512],
                        start=True, stop=True,
                    )
                    nc.vector.tensor_copy(FMT[96:97, nv * 512 : (nv + 1) * 512], ps2)


                # E^T blocks + s2 (column sums of E)
                ET = smp.tile([P, 8, NL], F32R, tag="ET")
                s2 = smp.tile([P, 8], F32, tag="s2")
                for tg in range(2):
                    ps = tp.tile([P, 512], F32R, tag="tp")
                    for tt in range(4):
                        t = tg * 4 + tt
                        nc.tensor.transpose(
                            ps[:, tt * P : (tt + 1) * P],
                            E[:, t * P : (t + 1) * P],
                            iden,
                        )
                    psv = ps.rearrange("p (four c) -> p four c", four=4)[:, :, :NL]
                    nc.scalar.activation(ET[:, tg * 4 : (tg + 1) * 4, :], psv, AF.Copy)
                    nc.vector.reduce_sum(
                        s2[:, tg * 4 : (tg + 1) * 4],
                        ET[:, tg * 4 : (tg + 1) * 4, :],
                        axis=mybir.AxisListType.X,
                    )
                ivs2 = smp.tile([P, 8], F32, tag="ivs2")
                nc.vector.reciprocal(ivs2, s2)

                if pending_finals:
                    pending_finals.pop(0)()

                # h1 = E @ fv -> [77, 768]; scaled by 1/s1 on copy-back
                psh = accp.tile([NL, DV], F32, tag="acc")
                for c0, cw in ((0, 512), (512, 256)):
                    sl = psh[:, c0 : c0 + cw]
                    for t in range(8):
                        nc.tensor.matmul(
                            sl, ET[:, t, :], FVn[:, t, c0 : c0 + cw],
                            start=(t == 0), stop=(t == 7),
                        )
                h1n = smp.tile([P, DV], F32R, tag="h1n")
                nc.scalar.activation(h1n[:NL, :], psh, AF.Identity, scale=ivs1)

                # H1T = h1n^T : [768, 77]
                H1T = smp.tile([P, 6, NL + 1], F32R, tag="H1T")
                for kg in range(2):
                    ps = tp.tile([P, 384], F32R, tag="tp")
                    for kk in range(3):
                        ko = kg * 3 + kk
                        nc.tensor.transpose(
                            ps[:, kk * P : (kk + 1) * P],
                            h1n[:, ko * P : (ko + 1) * P],
                            iden,
                        )
                    psv = ps.rearrange("p (three c) -> p three c", three=3)[:, :, : NL + 1]
                    if kg == 0:
                        nc.vector.tensor_copy(H1T[:, kg * 3 : (kg + 1) * 3, :], psv)
                    else:
                        nc.scalar.activation(H1T[:, kg * 3 : (kg + 1) * 3, :], psv, AF.Copy)

                if b + 1 < BL:
                    _emit_load_tg0(b + 1)

                if pending_finals:
                    pending_finals.pop(0)()

                # m_small = (fv_l @ W_vv^T) @ h1n^T + (fv_l @ b_vv) 1^T : [77, 78]
                MS = smp.tile([NL, NL + 1], F32R, tag="MS")
                psm = tp.tile([NL, NL + 1], F32, tag="tp")
                for ko in range(6):
                    nc.tensor.matmul(
                        psm, FWVT[:, ko, ls : ls + NL], H1T[:, ko, :],
                        start=(ko == 0), stop=False,
                    )
                nc.tensor.matmul(
                    psm, Cv[:1, ls : ls + NL], onesr[:1, : NL + 1],
                    start=False, stop=True,
                )
                nc.vector.tensor_copy(MS, psm)

                if pending_finals:
                    pending_finals.pop(0)()

                if b + 1 in vstate and len(vstate[b + 1][1]) == 1:
                    vstate[b + 1][1].append(_emit_tg(vstate[b + 1][0], 1))

                # fmT_un = m_small^T @ E : [77, 1024]; row 77 <- s2 row
                psf = accp.tile([NL, NV], F32, tag="acc")
                for nv in range(2):
                    nc.tensor.matmul(
                        psf[:, nv * 512 : (nv + 1) * 512],
                        MS[:, :NL], E[:NL, nv * 512 : (nv + 1) * 512],
                        start=True, stop=True,
                    )
                nc.vector.tensor_copy(FMT[:NL, :512], psf[:, :512])
                nc.scalar.activation(FMT[:NL, 512:], psf[:, 512:], AF.Copy)

                # finals for this batch are emitted during the NEXT batch
                # (software pipelining: their PE/copy/DMA work fills the
                # next batch's dependency stalls)
                def _emit_finals(b=b, FMT=FMT, ivs2=ivs2, ts=None):
                    for t in (ts if ts is not None else range(8)):
                        pso = accp.tile([P, OD], F32, tag="acc")
                        for c0, cw in ((0, 512), (512, 256)):
                            nc.tensor.matmul(
                                pso[:, c0 : c0 + cw],
                                FMT[:, t * P : (t + 1) * P],
                                Wm[:, c0 : c0 + cw],
                                start=True, stop=True,
                            )
                        OT = outp.tile([P, OD], F32, tag="OT")
                        if t % 2 == 0:
                            nc.vector.tensor_scalar_mul(OT, pso, ivs2[:, t, None])
                        else:
                            nc.scalar.activation(
                                OT, pso, AF.Identity, scale=ivs2[:, t, None]
                            )
                        nc.sync.dma_start(out_d[b, t * P : (t + 1) * P, :], OT)
                import functools as _ft
                pending_finals.append(_ft.partial(_emit_finals, ts=range(0, 3)))
                pending_finals.append(_ft.partial(_emit_finals, ts=range(3, 6)))
                pending_finals.append(_ft.partial(_emit_finals, ts=range(6, 8)))

                if b + 1 in vstate:
                    vstate[b + 1][2] = _emit_araw_E(
                        b + 1, vstate[b + 1][0], vstate[b + 1][1]
                    )

            for f in pending_finals:
                f()
            fvn3_pool.__exit__(None, None, None)

    nc.compile()
    return nc


_NC_CACHE = None
_last_in_maps = None


def _get_nc():
    global _NC_CACHE
    if _NC_CACHE is None:
        _NC_CACHE = _build()
    return _NC_CACHE


def kernel(**inputs) -> np.ndarray:
    fv = inputs["fv"]
    fl = inputs["fl"]
    consts = {
        "wkc": round_f32r(np.asarray(inputs["W_lk"]) @ np.asarray(inputs["W_vk"]).T),
        "wvc": round_f32r(np.asarray(inputs["W_lv"]) @ np.asarray(inputs["W_vv"]).T),
        "c2v": np.ascontiguousarray(
            np.asarray(inputs["W_vv"]) @ np.asarray(inputs["b_lv"]), dtype=np.float32
        ),
        "wcv": round_f32r(np.asarray(inputs["W_lv"]) @ np.asarray(inputs["b_vv"])),
        "ccv": np.array(
            [float(np.asarray(inputs["b_lv"]) @ np.asarray(inputs["b_vv"]))],
            dtype=np.float32,
        ),
        "wvkT": round_f32r(np.ascontiguousarray(inputs["W_vk"].T)),
        "wm": round_f32r(
            np.concatenate(
                [
                    np.asarray(inputs["W_m"]),
                    np.zeros((19, OD), np.float32),
                    np.asarray(inputs["b_m"])[None, :],
                ],
                axis=0,
            )
        ),
        "wc": round_f32r(np.asarray(inputs["W_lk"]) @ np.asarray(inputs["b_vk"])),
        "cc": np.array(
            [float(np.asarray(inputs["b_lk"]) @ np.asarray(inputs["b_vk"]))],
            dtype=np.float32,
        ),
        "c2": np.ascontiguousarray(
            np.asarray(inputs["W_vk"]) @ np.asarray(inputs["b_lk"]), dtype=np.float32
        ),
        "iden": np.eye(P, dtype=np.float32),
        "onesr": np.ones((1, 512), dtype=np.float32),
        "onesc": np.ones((P, 1), dtype=np.float32),
        "zeros": np.zeros((19, NV), dtype=np.float32),
    }
    fvr = round_f32r(fv)
    flr = round_f32r(fl)
    in_maps = []
    for c in range(NCORES):
        m = dict(consts)
        m["fv"] = np.ascontiguousarray(fvr[c * BL : (c + 1) * BL])
        m["fl"] = np.ascontiguousarray(flr[c * BL : (c + 1) * BL])
        in_maps.append(m)

    global _last_in_maps
    _last_in_maps = in_maps
    nc = _get_nc()
    res = run_bass_kernel_spmd(nc, in_maps, core_ids=list(range(NCORES)))
    out = np.concatenate([res.results[c]["out"] for c in range(NCORES)], axis=0)
    return np.ascontiguousarray(out, dtype=np.float32)
